# revision 5
# baseline (speedup 1.0000x reference)
"""AttentionBlock kernel for Trainium2, 8 NeuronCores.

Reference computation (B=4, C=256, H=W=64, TEMB=1024):
    t  = temb @ t_w.T + t_b                       # [B, C]
    q  = q_w @ x + (q_b + t)   (1x1 conv)         # [B, C, HW]
    k  = k_w @ x + (k_b + t)
    v  = v_w @ x + v_b
    att = softmax(q.T k / sqrt(C), axis=j)        # [B, HW, HW]
    hh  = att @ v.T                               # [B, C, HW]
    out = x + p_w @ hh + p_b

Sharding: data-parallel over (batch, query-half): core = b*2 + h.
Each core receives x[b] with its OWN query half rotated to the front
(keys may be processed in any order -- softmax is a sum over keys).
The flash-attention-style kernel runs entirely in SBUF: the 67M-entry
attention matrix is never materialized to HBM.

Algebraic folds (all exact, done on the host):
  - k bias (k_b + t): a per-channel shift of k adds a per-QUERY
    constant to every score row, which softmax normalizes away.
    Dropped.
  - v bias: softmax weights sum to 1, so hh = att@(v0+vb).T shifts by
    exactly vb; p_w @ vb joins p_b in the pre-biased residual.
  - temb projection: t = t_w@temb + t_b is a [B,C] host matvec; the
    device receives the finished q-bias vector.

Device-side layout (evidence-driven from NTFF traces):
  - everything up to the attention output runs in fp8e4 with
    MatmulPerfMode.DoubleRow (two 128-row k-tiles contracted per
    instruction at 2 rows/cycle).  Conv weights are pre-scaled by 64 so
    their ~0.02-scale entries land in fp8e4's normal range; q/k/v then
    carry a 64x scale that cancels in softmax (the 64s-column makes the
    denominator 64*sum(p)); the combined 1/(64*64*sqrt(C)) rides the
    exp instruction's input scale.
  - ONE unified 4KB-slot PSUM pool (2 bufs) serves the wide attention
    tiles, the conv psums, the transpose scratch and the p-conv psum;
    the other 4 banks hold the per-block mm2 accumulators.  Attention
    scores for a key-tile PAIR land in one [128,1024] psum tile so a
    single ACT exp instruction covers both (ACT throughput is the
    kernel's critical path; fewer instructions = less fixed overhead).
  - matmul 2: hhT[i, c] = sum_j p[j,i] vT[j,c] with fp8 p-slices as
    weights and fp8 vT pairs as the moving operand; vT's 64s-column
    makes hhT[:, 256] the softmax denominator.
  - normalization is a per-partition scale: DVE for blocks 0-2, ACT
    (idle by then) for the last block's tail.
  - hhT -> channel-major hh via bf16 PE transposes, 8 slices of one
    psum bank per block.
  - fp8 x+weights ride the sync DMA queue (first conv starts ~1us in);
    block 0's attention interleaves with the convs chunk-by-chunk,
    q-convs for block N are deferred to block N's start, and block N's
    p-conv is deferred into block N+1's matmul stream.
"""

import numpy as np
import ml_dtypes
from contextlib import ExitStack

import concourse.bass as bass
import concourse.mybir as mybir
import concourse.tile as tile
from concourse import bacc
from concourse.bass_utils import run_bass_kernel_spmd

F32 = mybir.dt.float32
BF16 = mybir.dt.bfloat16
FP8 = mybir.dt.float8e4
AF = mybir.ActivationFunctionType
DR = mybir.MatmulPerfMode.DoubleRow

B, C, H, W, TEMB = 4, 256, 64, 64, 1024
HW = H * W              # 4096
NQ = HW // 2            # 2048 query pixels per core
N_CORES = 8
WS = 64.0               # fp8 weight pre-scale
ESCALE = (float(C) ** -0.5) / (WS * WS)   # rides the exp instruction

N_CH = HW // 512        # 8 x-chunks of 512 columns
N_JT = HW // 128        # 32 key tiles of 128
N_PAIR = N_JT // 2      # 16 key-tile pairs (DoubleRow contracts 2 at once)
N_IB = NQ // 512        # 4 query blocks of 512
N_KT = C // 128         # 2 channel tiles
CV = C + 1              # vT width: 256 channels + 64s col


def build_nc():
    nc = bacc.Bacc("TRN2", target_bir_lowering=False, debug=False)

    # ---- DRAM I/O (per-core) ----
    x8_d = nc.dram_tensor("x8", [N_CH * C, 512], FP8, kind="ExternalInput")
    xr_d = nc.dram_tensor("xr", [C, NQ], F32, kind="ExternalInput")
    qwT_d = nc.dram_tensor("qwT", [C, C], FP8, kind="ExternalInput")
    kwT_d = nc.dram_tensor("kwT", [C, C], FP8, kind="ExternalInput")
    vwT_d = nc.dram_tensor("vwT", [C, C], FP8, kind="ExternalInput")
    pwT_d = nc.dram_tensor("pwT", [C, C], BF16, kind="ExternalInput")
    qb_d = nc.dram_tensor("qb", [C, 1], F32, kind="ExternalInput")
    id_d = nc.dram_tensor("ident", [128, 128], BF16, kind="ExternalInput")
    out_d = nc.dram_tensor("out", [C, NQ], F32, kind="ExternalOutput")

    with tile.TileContext(nc) as tc, ExitStack() as ctx:
        const = ctx.enter_context(tc.tile_pool(name="const", bufs=1))
        big = ctx.enter_context(tc.tile_pool(name="big", bufs=1))

        def load3(dram, width, name, eng):
            t = const.tile([128, N_KT, width], FP8, tag=name)
            eng.dma_start(out=t, in_=dram[:].rearrange("(a p) o -> p a o", p=128))
            return t

        # queue A (sync): the fp8 conv stream, then the f32 residual
        kwT = load3(kwT_d, C, "kwT", nc.sync)
        qwT = load3(qwT_d, C, "qwT", nc.sync)
        vwT = load3(vwT_d, C, "vwT", nc.sync)
        qb = const.tile([128, N_KT], F32, tag="qb")
        nc.sync.dma_start(
            out=qb, in_=qb_d[:].rearrange("(a p) one -> p (a one)", p=128))
        x8_ch = [big.tile([128, N_KT, 512], FP8, tag=f"x8_{ch}", name=f"x8_{ch}")
                 for ch in range(N_CH)]
        for ch in range(N_CH):
            nc.sync.dma_start(
                out=x8_ch[ch],
                in_=x8_d[ch * C:(ch + 1) * C, :].rearrange(
                    "(a p) o -> p a o", p=128),
            )
        xr_ch = [[big.tile([128, 512], F32, tag=f"xr{kt}_{ib}",
                           name=f"xr_{kt}_{ib}") for ib in range(N_IB)]
                 for kt in range(N_KT)]
        for ib in range(N_IB):
            for kt in range(N_KT):
                nc.sync.dma_start(
                    out=xr_ch[kt][ib],
                    in_=xr_d[kt * 128:(kt + 1) * 128, ib * 512:(ib + 1) * 512])

        # queue B (gpsimd): tail constants
        pwT = const.tile([128, N_KT, C], BF16, tag="pwT")
        nc.gpsimd.dma_start(
            out=pwT, in_=pwT_d[:].rearrange("(a p) o -> p a o", p=128))
        ident = const.tile([128, 128], BF16, tag="ident")
        nc.gpsimd.dma_start(out=ident, in_=id_d[:])

        # per-chunk K / V~T / Q tiles (fp8) for fine-grained dependencies
        k_ch = [big.tile([128, N_KT, 512], FP8, tag=f"k{ch}", name=f"k_{ch}")
                for ch in range(N_CH)]
        vT_ch = [big.tile([128, 4, CV], FP8, tag=f"vT{ch}", name=f"vT_{ch}")
                 for ch in range(N_CH)]
        q_ch = [big.tile([128, N_KT, 512], FP8, tag=f"q{ib}", name=f"q_{ib}")
                for ib in range(N_IB)]
        # the 64s denominator column (the Pool engine owns SBUF memsets)
        for ch in range(N_CH):
            nc.gpsimd.memset(vT_ch[ch][:, :, C:C + 1], WS)

        with tc.tile_pool(name="P1", bufs=2, space="PSUM") as P1, \
             tc.tile_pool(name="hps", bufs=4, space="PSUM") as hps, \
             tc.tile_pool(name="ppool", bufs=4) as ppool, \
             tc.tile_pool(name="htpool", bufs=6) as htpool, \
             tc.tile_pool(name="hhpool", bufs=4) as hhpool, \
             tc.tile_pool(name="opool", bufs=4) as opool, \
             tc.tile_pool(name="rpool", bufs=8) as rpool:

            hh_ps_of = {}

            def emit_qconv(ib):
                ps = P1.tile([128, 2, 512], F32, tag="w", name=f"psq{ib}")
                for mt in range(N_KT):
                    nc.tensor.matmul(
                        ps[:, mt, :],
                        lhsT=qwT[:, :, mt * 128:(mt + 1) * 128],
                        rhs=x8_ch[ib][:, :, :],
                        start=True, stop=True, perf_mode=DR,
                    )
                for mt in range(N_KT):
                    nc.vector.tensor_scalar_add(
                        q_ch[ib][:, mt, :], ps[:, mt, :], qb[:, mt:mt + 1])

            def emit_attn_pair(ib, pair):
                """One key-tile PAIR of attention for query block ib."""
                ch, pp = divmod(pair, 2)
                att = P1.tile([128, 2, 512], F32, tag="w",
                              name=f"att{ib}_{pair}")
                for h in range(2):
                    jj = pp * 2 + h
                    nc.tensor.matmul(
                        att[:, h, :],
                        lhsT=k_ch[ch][:, :, jj * 128:(jj + 1) * 128],
                        rhs=q_ch[ib][:, :, :],
                        start=True, stop=True, perf_mode=DR,
                    )
                pt = ppool.tile([128, 2, 512], FP8, tag="pT",
                                name=f"pt{ib}_{pair}")
                nc.scalar.activation(out=pt[:, :, :], in_=att[:, :, :],
                                     func=AF.Exp, scale=ESCALE)
                for isl in range(4):
                    nc.tensor.matmul(
                        hh_ps_of[ib][isl],
                        lhsT=pt[:, :, isl * 128:(isl + 1) * 128],
                        rhs=vT_ch[ch][:, pp * 2:pp * 2 + 2, :],
                        start=(pair == 0),
                        stop=(pair == N_PAIR - 1),
                        perf_mode=DR,
                    )

            def emit_tail(ib, on_act):
                """Normalize + transpose to channel-major for block ib."""
                hh_half = [hhpool.tile([128, 512], BF16, tag=f"hhsb{ct}",
                                       name=f"hh_half{ib}_{ct}")
                           for ct in range(N_KT)]
                trp = P1.tile([128, 16, 128], BF16, tag="w", name=f"trp{ib}")
                for isl in range(4):
                    rc = rpool.tile([128, 1], F32, tag="rc", name=f"rc{ib}_{isl}")
                    nc.vector.reciprocal(rc, hh_ps_of[ib][isl][:, C:C + 1])
                    ht = htpool.tile([128, C], BF16, tag="ht", name=f"ht{ib}_{isl}")
                    if on_act:
                        nc.scalar.activation(out=ht, in_=hh_ps_of[ib][isl][:, 0:C],
                                             func=AF.Copy, scale=rc)
                    else:
                        nc.vector.tensor_scalar_mul(
                            ht, hh_ps_of[ib][isl][:, 0:C], rc)
                    for ct in range(N_KT):
                        s = isl * 2 + ct
                        nc.tensor.transpose(
                            trp[:, s, :], ht[:, ct * 128:(ct + 1) * 128], ident)
                        nc.vector.tensor_copy(
                            hh_half[ct][:, isl * 128:(isl + 1) * 128],
                            trp[:, s, :])
                return hh_half

            def emit_pconv(ib, hh_half):
                """p-conv + residual (pre-biased on host) + store."""
                i0 = ib * 512
                po = P1.tile([128, 2, 512], F32, tag="w", name=f"po{ib}")
                for ot in range(N_KT):
                    for ct in range(N_KT):
                        nc.tensor.matmul(
                            po[:, ot, :],
                            lhsT=pwT[:, ct, ot * 128:(ot + 1) * 128],
                            rhs=hh_half[ct],
                            start=(ct == 0),
                            stop=(ct == N_KT - 1),
                        )
                    ob = opool.tile([128, 512], F32, tag="ob", name=f"ob{ib}_{ot}")
                    nc.vector.tensor_add(ob, po[:, ot, :], xr_ch[ot][ib])
                    nc.sync.dma_start(
                        out=out_d[ot * 128:(ot + 1) * 128, i0:i0 + 512], in_=ob
                    )

            # ---- conv phase, interleaved with query block 0's attention ----
            hh_ps_of[0] = [hps.tile([128, CV], F32, tag="hh",
                                    name=f"hh_ps0_{isl}") for isl in range(4)]
            emit_qconv(0)
            for ch in range(N_CH):
                ps = P1.tile([128, 2, 512], F32, tag="w", name=f"psk{ch}")
                for mt in range(N_KT):
                    nc.tensor.matmul(
                        ps[:, mt, :],
                        lhsT=kwT[:, :, mt * 128:(mt + 1) * 128],
                        rhs=x8_ch[ch][:, :, :],
                        start=True, stop=True, perf_mode=DR,
                    )
                nc.vector.tensor_copy(k_ch[ch][:, :, :], ps[:, :, :])
                for jjh in range(2):
                    ps = P1.tile([128, 2, 512], F32, tag="w",
                                 name=f"psv{ch}_{jjh}")
                    for h in range(2):
                        jj = jjh * 2 + h
                        nc.tensor.matmul(
                            ps[:, h, 0:C],
                            lhsT=x8_ch[ch][:, :, jj * 128:(jj + 1) * 128],
                            rhs=vwT[:, :, :],
                            start=True, stop=True, perf_mode=DR,
                        )
                    nc.vector.tensor_copy(
                        vT_ch[ch][:, jjh * 2:jjh * 2 + 2, 0:C], ps[:, :, 0:C])
                # trail one chunk behind the convs with block 0's attention
                if ch >= 1:
                    for pair in range(2 * (ch - 1), 2 * ch):
                        emit_attn_pair(0, pair)
            for pair in range(2 * (N_CH - 1), N_PAIR):
                emit_attn_pair(0, pair)

            # ---- remaining query blocks ----
            pending = (0, emit_tail(0, on_act=False))
            for ib in range(1, N_IB):
                emit_qconv(ib)
                hh_ps_of[ib] = [hps.tile([128, CV], F32, tag="hh",
                                         name=f"hh_ps{ib}_{isl}")
                                for isl in range(4)]
                for pair in range(N_PAIR):
                    emit_attn_pair(ib, pair)
                    if pending is not None and pair == 2:
                        emit_pconv(pending[0], pending[1])
                        pending = None
                pending = (ib, emit_tail(ib, on_act=(ib == N_IB - 1)))
            emit_pconv(pending[0], pending[1])

    nc.compile()
    return nc


_NC_CACHE = None


def _get_nc():
    global _NC_CACHE
    if _NC_CACHE is None:
        _NC_CACHE = build_nc()
    return _NC_CACHE


def make_in_maps(x, temb, q_w, q_b, k_w, k_b, v_w, v_b, p_w, p_b, t_w, t_b):
    xf = np.asarray(x, np.float32).reshape(B, C, HW)
    temb = np.asarray(temb, np.float32)
    bf16 = ml_dtypes.bfloat16
    fp8 = ml_dtypes.float8_e4m3
    # host-side algebraic folds
    t = temb @ np.asarray(t_w, np.float32).T + np.asarray(t_b, np.float32)
    rbias = (np.asarray(p_b, np.float32)
             + np.asarray(p_w, np.float32) @ np.asarray(v_b, np.float32))
    common = {
        "qwT": np.ascontiguousarray(np.asarray(q_w, np.float32).T * WS).astype(fp8),
        "kwT": np.ascontiguousarray(np.asarray(k_w, np.float32).T * WS).astype(fp8),
        "vwT": np.ascontiguousarray(np.asarray(v_w, np.float32).T * WS).astype(fp8),
        "pwT": np.ascontiguousarray(np.asarray(p_w, np.float32).T).astype(bf16),
        "ident": np.eye(128, dtype=bf16),
    }
    in_maps = []
    for core in range(N_CORES):
        b, h = divmod(core, 2)
        m = dict(common)
        # rotate so this core's query half occupies columns 0..NQ-1;
        # key order is irrelevant (softmax sums over keys).
        xr = xf[b] if h == 0 else np.concatenate(
            [xf[b][:, NQ:], xf[b][:, :NQ]], axis=1)
        # chunk-contiguous: [N_CH, C, 512] flattened, so each 128KB chunk
        # is one linear DRAM span (fast DMA descriptors)
        m["x8"] = np.ascontiguousarray(
            xr.reshape(C, N_CH, 512).transpose(1, 0, 2)).reshape(
                N_CH * C, 512).astype(fp8)
        m["xr"] = xr[:, :NQ] + rbias[:, None]
        m["qb"] = (WS * (np.asarray(q_b, np.float32) + t[b])).reshape(C, 1)
        in_maps.append(m)
    return in_maps


def run(in_maps, trace=False):
    nc = _get_nc()
    return run_bass_kernel_spmd(nc, in_maps, core_ids=list(range(N_CORES)),
                                trace=trace)


def kernel(**inputs):
    in_maps = make_in_maps(**inputs)
    res = run(in_maps)
    out = np.empty((B, C, HW), np.float32)
    for core in range(N_CORES):
        b, h = divmod(core, 2)
        out[b, :, h * NQ:(h + 1) * NQ] = res.results[core]["out"]
    return out.reshape(B, C, H, W)


# revision 6
# speedup vs baseline: 1.2131x; 1.2131x over previous
"""AttentionBlock kernel for Trainium2, 8 NeuronCores.

Reference computation (B=4, C=256, H=W=64, TEMB=1024):
    t  = temb @ t_w.T + t_b                       # [B, C]
    q  = q_w @ x + (q_b + t)   (1x1 conv)         # [B, C, HW]
    k  = k_w @ x + (k_b + t)
    v  = v_w @ x + v_b
    att = softmax(q.T k / sqrt(C), axis=j)        # [B, HW, HW]
    hh  = att @ v.T                               # [B, C, HW]
    out = x + p_w @ hh + p_b

Sharding: data-parallel over (batch, query-half): core = b*2 + h.
Each core receives x[b] with its OWN query half rotated to the front
(keys may be processed in any order -- softmax is a sum over keys).
The kernel runs entirely in SBUF: the 67M-entry attention matrix is
never materialized to HBM.

Algebraic folds (all exact, done on the host):
  - k bias (k_b + t): a per-channel shift of k adds a per-QUERY
    constant to every score row, which softmax normalizes away. Gone.
  - v bias: softmax weights sum to 1, so it passes straight through
    the attention average; p_w @ v_b joins p_b in the residual.
  - temb projection: a [B,C] host matvec; the device receives the
    finished q-bias vector.
  - THE BIG ONE: p_w folds into the v conv.  W = p_w @ v_w (host), so
    matmul 2 accumulates sum_j p[j,i] * (W x)[o,j] -- the attention
    BLOCK output directly.  The p-conv, the i->c transposes and their
    staging all vanish; the tail is one fused (hh*rc)+residual DVE op
    per 128-query slice, stored i-major (the host un-transposes).

Device-side layout (evidence-driven from NTFF traces):
  - everything runs in fp8e4 with MatmulPerfMode.DoubleRow (two
    128-row k-tiles contracted per instruction at 2 rows/cycle).
    Conv weights are pre-scaled by 64 so their ~0.02-scale entries
    land in fp8e4's normal range; q/k/w then carry a 64x scale that
    cancels in softmax (the 64s-column makes the denominator
    64*sum(p)); the combined 1/(64*64*sqrt(C)) rides the exp
    instruction's input scale.
  - ONE unified 4KB-slot PSUM pool (2 bufs) serves the wide attention
    tiles and the conv psums; the other 4 banks hold the per-block
    mm2 accumulators.  Scores for a key-tile PAIR land in one
    [128,1024] psum tile so a single ACT exp instruction covers both
    (fewer instructions = less fixed overhead on the ACT, which
    otherwise paces the kernel).
  - fp8 x+weights ride the sync DMA queue (first conv starts ~1us
    in); block 0's attention interleaves with the convs
    chunk-by-chunk and q-convs for block N are deferred to block N's
    start so the early DVE copy stream stays short.
"""

import numpy as np
import ml_dtypes
from contextlib import ExitStack

import concourse.bass as bass
import concourse.mybir as mybir
import concourse.tile as tile
from concourse import bacc
from concourse.bass_utils import run_bass_kernel_spmd

F32 = mybir.dt.float32
BF16 = mybir.dt.bfloat16
FP8 = mybir.dt.float8e4
AF = mybir.ActivationFunctionType
DR = mybir.MatmulPerfMode.DoubleRow

B, C, H, W, TEMB = 4, 256, 64, 64, 1024
HW = H * W              # 4096
NQ = HW // 2            # 2048 query pixels per core
N_CORES = 8
WS = 64.0               # fp8 weight pre-scale
ESCALE = (float(C) ** -0.5) / (WS * WS)   # rides the exp instruction

N_CH = HW // 512        # 8 x-chunks of 512 columns
N_JT = HW // 128        # 32 key tiles of 128
N_PAIR = N_JT // 2      # 16 key-tile pairs (DoubleRow contracts 2 at once)
N_IB = NQ // 512        # 4 query blocks of 512
N_KT = C // 128         # 2 channel tiles
CV = C + 1              # wT width: 256 out-channels + 64s col


def build_nc():
    nc = bacc.Bacc("TRN2", target_bir_lowering=False, debug=False)

    # ---- DRAM I/O (per-core) ----
    x8_d = nc.dram_tensor("x8", [N_CH * C, 512], FP8, kind="ExternalInput")
    xr_d = nc.dram_tensor("xr", [NQ, C], F32, kind="ExternalInput")
    qwT_d = nc.dram_tensor("qwT", [C, C], FP8, kind="ExternalInput")
    kwT_d = nc.dram_tensor("kwT", [C, C], FP8, kind="ExternalInput")
    wwT_d = nc.dram_tensor("wwT", [C, C], FP8, kind="ExternalInput")
    qb_d = nc.dram_tensor("qb", [C, 1], F32, kind="ExternalInput")
    out_d = nc.dram_tensor("out", [NQ, C], F32, kind="ExternalOutput")

    with tile.TileContext(nc) as tc, ExitStack() as ctx:
        const = ctx.enter_context(tc.tile_pool(name="const", bufs=1))
        big = ctx.enter_context(tc.tile_pool(name="big", bufs=1))

        def load3(dram, name):
            t = const.tile([128, N_KT, C], FP8, tag=name)
            nc.sync.dma_start(
                out=t, in_=dram[:].rearrange("(a p) o -> p a o", p=128))
            return t

        # one DMA queue: fp8 weights, fp8 x chunks, then the residual
        kwT = load3(kwT_d, "kwT")
        qwT = load3(qwT_d, "qwT")
        wwT = load3(wwT_d, "wwT")
        qb = const.tile([128, N_KT], F32, tag="qb")
        nc.sync.dma_start(
            out=qb, in_=qb_d[:].rearrange("(a p) one -> p (a one)", p=128))
        x8_ch = [big.tile([128, N_KT, 512], FP8, tag=f"x8_{ch}", name=f"x8_{ch}")
                 for ch in range(N_CH)]
        for ch in range(N_CH):
            nc.sync.dma_start(
                out=x8_ch[ch],
                in_=x8_d[ch * C:(ch + 1) * C, :].rearrange(
                    "(a p) o -> p a o", p=128),
            )
        xr_ib = [big.tile([128, 4, C], F32, tag=f"xr{ib}", name=f"xr_{ib}")
                 for ib in range(N_IB)]
        for ib in range(N_IB):
            nc.sync.dma_start(
                out=xr_ib[ib],
                in_=xr_d[ib * 512:(ib + 1) * 512, :].rearrange(
                    "(a p) o -> p a o", p=128),
            )

        # per-chunk K / W~T / Q tiles (fp8) for fine-grained dependencies
        k_ch = [big.tile([128, N_KT, 512], FP8, tag=f"k{ch}", name=f"k_{ch}")
                for ch in range(N_CH)]
        wT_ch = [big.tile([128, 4, CV], FP8, tag=f"wT{ch}", name=f"wT_{ch}")
                 for ch in range(N_CH)]
        q_ch = [big.tile([128, N_KT, 512], FP8, tag=f"q{ib}", name=f"q_{ib}")
                for ib in range(N_IB)]
        # the 64s denominator column (the Pool engine owns SBUF memsets)
        for ch in range(N_CH):
            nc.gpsimd.memset(wT_ch[ch][:, :, C:C + 1], WS)

        with tc.tile_pool(name="P1", bufs=2, space="PSUM") as P1, \
             tc.tile_pool(name="hps", bufs=4, space="PSUM") as hps, \
             tc.tile_pool(name="ppool", bufs=4) as ppool, \
             tc.tile_pool(name="opool", bufs=6) as opool, \
             tc.tile_pool(name="rpool", bufs=8) as rpool:

            hh_ps_of = {}

            def emit_qconv(ib):
                ps = P1.tile([128, 2, 512], F32, tag="w", name=f"psq{ib}")
                for mt in range(N_KT):
                    nc.tensor.matmul(
                        ps[:, mt, :],
                        lhsT=qwT[:, :, mt * 128:(mt + 1) * 128],
                        rhs=x8_ch[ib][:, :, :],
                        start=True, stop=True, perf_mode=DR,
                    )
                for mt in range(N_KT):
                    nc.vector.tensor_scalar_add(
                        q_ch[ib][:, mt, :], ps[:, mt, :], qb[:, mt:mt + 1])

            def emit_attn_pair(ib, pair):
                """One key-tile PAIR of attention for query block ib."""
                ch, pp = divmod(pair, 2)
                att = P1.tile([128, 2, 512], F32, tag="w",
                              name=f"att{ib}_{pair}")
                for h in range(2):
                    jj = pp * 2 + h
                    nc.tensor.matmul(
                        att[:, h, :],
                        lhsT=k_ch[ch][:, :, jj * 128:(jj + 1) * 128],
                        rhs=q_ch[ib][:, :, :],
                        start=True, stop=True, perf_mode=DR,
                    )
                pt = ppool.tile([128, 2, 512], FP8, tag="pT",
                                name=f"pt{ib}_{pair}")
                nc.scalar.activation(out=pt[:, :, :], in_=att[:, :, :],
                                     func=AF.Exp, scale=ESCALE)
                for isl in range(4):
                    nc.tensor.matmul(
                        hh_ps_of[ib][isl],
                        lhsT=pt[:, :, isl * 128:(isl + 1) * 128],
                        rhs=wT_ch[ch][:, pp * 2:pp * 2 + 2, :],
                        start=(pair == 0),
                        stop=(pair == N_PAIR - 1),
                        perf_mode=DR,
                    )

            def emit_tail(ib):
                """Normalize + residual + store, i-major, for block ib."""
                for isl in range(4):
                    rc = rpool.tile([128, 1], F32, tag="rc", name=f"rc{ib}_{isl}")
                    nc.vector.reciprocal(rc, hh_ps_of[ib][isl][:, C:C + 1])
                    ob = opool.tile([128, C], F32, tag="ob",
                                    name=f"ob{ib}_{isl}")
                    nc.vector.scalar_tensor_tensor(
                        ob, in0=hh_ps_of[ib][isl][:, 0:C], scalar=rc,
                        in1=xr_ib[ib][:, isl, :],
                        op0=mybir.AluOpType.mult, op1=mybir.AluOpType.add,
                    )
                    r0 = ib * 512 + isl * 128
                    nc.sync.dma_start(out=out_d[r0:r0 + 128, :], in_=ob)

            # ---- conv phase, interleaved with query block 0's attention ----
            hh_ps_of[0] = [hps.tile([128, CV], F32, tag="hh",
                                    name=f"hh_ps0_{isl}") for isl in range(4)]
            emit_qconv(0)
            for ch in range(N_CH):
                ps = P1.tile([128, 2, 512], F32, tag="w", name=f"psk{ch}")
                for mt in range(N_KT):
                    nc.tensor.matmul(
                        ps[:, mt, :],
                        lhsT=kwT[:, :, mt * 128:(mt + 1) * 128],
                        rhs=x8_ch[ch][:, :, :],
                        start=True, stop=True, perf_mode=DR,
                    )
                nc.vector.tensor_copy(k_ch[ch][:, :, :], ps[:, :, :])
                for jjh in range(2):
                    ps = P1.tile([128, 2, 512], F32, tag="w",
                                 name=f"psw{ch}_{jjh}")
                    for h in range(2):
                        jj = jjh * 2 + h
                        nc.tensor.matmul(
                            ps[:, h, 0:C],
                            lhsT=x8_ch[ch][:, :, jj * 128:(jj + 1) * 128],
                            rhs=wwT[:, :, :],
                            start=True, stop=True, perf_mode=DR,
                        )
                    nc.vector.tensor_copy(
                        wT_ch[ch][:, jjh * 2:jjh * 2 + 2, 0:C], ps[:, :, 0:C])
                # trail one chunk behind the convs with block 0's attention
                if ch >= 1:
                    for pair in range(2 * (ch - 1), 2 * ch):
                        emit_attn_pair(0, pair)
            for pair in range(2 * (N_CH - 1), N_PAIR):
                emit_attn_pair(0, pair)

            # ---- remaining query blocks ----
            pending = 0
            for ib in range(1, N_IB):
                emit_qconv(ib)
                hh_ps_of[ib] = [hps.tile([128, CV], F32, tag="hh",
                                         name=f"hh_ps{ib}_{isl}")
                                for isl in range(4)]
                for pair in range(N_PAIR):
                    emit_attn_pair(ib, pair)
                    if pending is not None and pair == 1:
                        emit_tail(pending)
                        pending = None
                pending = ib
            emit_tail(pending)

    nc.compile()
    return nc


_NC_CACHE = None


def _get_nc():
    global _NC_CACHE
    if _NC_CACHE is None:
        _NC_CACHE = build_nc()
    return _NC_CACHE


def make_in_maps(x, temb, q_w, q_b, k_w, k_b, v_w, v_b, p_w, p_b, t_w, t_b):
    xf = np.asarray(x, np.float32).reshape(B, C, HW)
    temb = np.asarray(temb, np.float32)
    fp8 = ml_dtypes.float8_e4m3
    # host-side algebraic folds
    t = temb @ np.asarray(t_w, np.float32).T + np.asarray(t_b, np.float32)
    pw = np.asarray(p_w, np.float32)
    Ww = pw @ np.asarray(v_w, np.float32)
    rbias = np.asarray(p_b, np.float32) + pw @ np.asarray(v_b, np.float32)
    common = {
        "qwT": np.ascontiguousarray(np.asarray(q_w, np.float32).T * WS).astype(fp8),
        "kwT": np.ascontiguousarray(np.asarray(k_w, np.float32).T * WS).astype(fp8),
        "wwT": np.ascontiguousarray(Ww.T * WS).astype(fp8),
    }
    in_maps = []
    for core in range(N_CORES):
        b, h = divmod(core, 2)
        m = dict(common)
        # rotate so this core's query half occupies columns 0..NQ-1;
        # key order is irrelevant (softmax sums over keys).
        xr = xf[b] if h == 0 else np.concatenate(
            [xf[b][:, NQ:], xf[b][:, :NQ]], axis=1)
        # chunk-contiguous: [N_CH, C, 512] flattened, so each 128KB chunk
        # is one linear DRAM span (fast DMA descriptors)
        m["x8"] = np.ascontiguousarray(
            xr.reshape(C, N_CH, 512).transpose(1, 0, 2)).reshape(
                N_CH * C, 512).astype(fp8)
        # residual, pre-biased and transposed to the i-major store layout
        m["xr"] = np.ascontiguousarray((xr[:, :NQ] + rbias[:, None]).T)
        m["qb"] = (WS * (np.asarray(q_b, np.float32) + t[b])).reshape(C, 1)
        in_maps.append(m)
    return in_maps


def run(in_maps, trace=False):
    nc = _get_nc()
    return run_bass_kernel_spmd(nc, in_maps, core_ids=list(range(N_CORES)),
                                trace=trace)


def kernel(**inputs):
    in_maps = make_in_maps(**inputs)
    res = run(in_maps)
    out = np.empty((B, C, HW), np.float32)
    for core in range(N_CORES):
        b, h = divmod(core, 2)
        out[b, :, h * NQ:(h + 1) * NQ] = res.results[core]["out"].T
    return out.reshape(B, C, H, W)


# revision 8
# speedup vs baseline: 1.2141x; 1.0009x over previous
"""AttentionBlock kernel for Trainium2, 8 NeuronCores.

Reference computation (B=4, C=256, H=W=64, TEMB=1024):
    t  = temb @ t_w.T + t_b                       # [B, C]
    q  = q_w @ x + (q_b + t)   (1x1 conv)         # [B, C, HW]
    k  = k_w @ x + (k_b + t)
    v  = v_w @ x + v_b
    att = softmax(q.T k / sqrt(C), axis=j)        # [B, HW, HW]
    hh  = att @ v.T                               # [B, C, HW]
    out = x + p_w @ hh + p_b

Sharding: data-parallel over (batch, query-half): core = b*2 + h.
Each core receives x[b] with its OWN query half rotated to the front
(keys may be processed in any order -- softmax is a sum over keys).
The kernel runs entirely in SBUF: the 67M-entry attention matrix is
never materialized to HBM.

Algebraic folds (all exact, done on the host):
  - k bias (k_b + t): a per-channel shift of k adds a per-QUERY
    constant to every score row, which softmax normalizes away. Gone.
  - v bias: softmax weights sum to 1, so it passes straight through
    the attention average; p_w @ v_b joins p_b in the residual.
  - temb projection: a [B,C] host matvec; the device receives the
    finished q-bias vector.
  - THE BIG ONE: p_w folds into the v conv.  W = p_w @ v_w (host), so
    matmul 2 accumulates sum_j p[j,i] * (W x)[o,j] -- the attention
    BLOCK output directly.  The p-conv, the i->c transposes and their
    staging all vanish; the tail is one fused (hh*rc)+residual DVE op
    per 128-query slice, stored i-major (the host un-transposes).

Device-side layout (evidence-driven from NTFF traces):
  - everything runs in fp8e4 with MatmulPerfMode.DoubleRow (two
    128-row k-tiles contracted per instruction at 2 rows/cycle).
    Conv weights are pre-scaled by 64 so their ~0.02-scale entries
    land in fp8e4's normal range; q/k/w then carry a 64x scale that
    cancels in softmax (the 64s-column makes the denominator
    64*sum(p)); the combined 1/(64*64*sqrt(C)) rides the exp
    instruction's input scale.
  - ONE unified 4KB-slot PSUM pool (2 bufs) serves the wide attention
    tiles and the conv psums; the other 4 banks hold the per-block
    mm2 accumulators.  Scores for a key-tile PAIR land in one
    [128,1024] psum tile so a single ACT exp instruction covers both
    (fewer instructions = less fixed overhead on the ACT, which
    otherwise paces the kernel).
  - fp8 x+weights ride the sync DMA queue (first conv starts ~1us
    in); block 0's attention interleaves with the convs
    chunk-by-chunk and q-convs for block N are deferred to block N's
    start so the early DVE copy stream stays short.
"""

import numpy as np
import ml_dtypes
from contextlib import ExitStack

import concourse.bass as bass
import concourse.mybir as mybir
import concourse.tile as tile
from concourse import bacc
from concourse.bass_utils import run_bass_kernel_spmd

F32 = mybir.dt.float32
BF16 = mybir.dt.bfloat16
FP8 = mybir.dt.float8e4
AF = mybir.ActivationFunctionType
DR = mybir.MatmulPerfMode.DoubleRow

B, C, H, W, TEMB = 4, 256, 64, 64, 1024
HW = H * W              # 4096
NQ = HW // 2            # 2048 query pixels per core
N_CORES = 8
WS = 64.0               # fp8 weight pre-scale
ESCALE = (float(C) ** -0.5) / (WS * WS)   # rides the exp instruction

N_CH = HW // 512        # 8 x-chunks of 512 columns
N_JT = HW // 128        # 32 key tiles of 128
N_PAIR = N_JT // 2      # 16 key-tile pairs (DoubleRow contracts 2 at once)
N_IB = NQ // 512        # 4 query blocks of 512
N_KT = C // 128         # 2 channel tiles
CV = C + 1              # wT width: 256 out-channels + 64s col


def build_nc():
    nc = bacc.Bacc("TRN2", target_bir_lowering=False, debug=False)

    # ---- DRAM I/O (per-core) ----
    x8_d = nc.dram_tensor("x8", [N_CH * C, 512], FP8, kind="ExternalInput")
    xr_d = nc.dram_tensor("xr", [NQ, C], F32, kind="ExternalInput")
    qwT_d = nc.dram_tensor("qwT", [C, C], FP8, kind="ExternalInput")
    kwT_d = nc.dram_tensor("kwT", [C, C], FP8, kind="ExternalInput")
    wwT_d = nc.dram_tensor("wwT", [C, C], FP8, kind="ExternalInput")
    qb_d = nc.dram_tensor("qb", [C, 1], F32, kind="ExternalInput")
    out_d = nc.dram_tensor("out", [NQ, C], F32, kind="ExternalOutput")

    with tile.TileContext(nc) as tc, ExitStack() as ctx:
        const = ctx.enter_context(tc.tile_pool(name="const", bufs=1))
        big = ctx.enter_context(tc.tile_pool(name="big", bufs=1))

        def load3(dram, name):
            t = const.tile([128, N_KT, C], FP8, tag=name)
            nc.sync.dma_start(
                out=t, in_=dram[:].rearrange("(a p) o -> p a o", p=128))
            return t

        # one DMA queue: fp8 weights, fp8 x chunks, then the residual
        kwT = load3(kwT_d, "kwT")
        qwT = load3(qwT_d, "qwT")
        wwT = load3(wwT_d, "wwT")
        qb = const.tile([128, N_KT], F32, tag="qb")
        nc.sync.dma_start(
            out=qb, in_=qb_d[:].rearrange("(a p) one -> p (a one)", p=128))
        x8_ch = [big.tile([128, N_KT, 512], FP8, tag=f"x8_{ch}", name=f"x8_{ch}")
                 for ch in range(N_CH)]
        for ch in range(N_CH):
            nc.sync.dma_start(
                out=x8_ch[ch],
                in_=x8_d[ch * C:(ch + 1) * C, :].rearrange(
                    "(a p) o -> p a o", p=128),
            )
        xr_ib = [big.tile([128, 4, C], F32, tag=f"xr{ib}", name=f"xr_{ib}")
                 for ib in range(N_IB)]
        for ib in range(N_IB):
            nc.sync.dma_start(
                out=xr_ib[ib],
                in_=xr_d[ib * 512:(ib + 1) * 512, :].rearrange(
                    "(a p) o -> p a o", p=128),
            )

        # per-chunk K / W~T / Q tiles (fp8) for fine-grained dependencies
        k_ch = [big.tile([128, N_KT, 512], FP8, tag=f"k{ch}", name=f"k_{ch}")
                for ch in range(N_CH)]
        wT_ch = [big.tile([128, 4, CV], FP8, tag=f"wT{ch}", name=f"wT_{ch}")
                 for ch in range(N_CH)]
        q_ch = [big.tile([128, N_KT, 512], FP8, tag=f"q{ib}", name=f"q_{ib}")
                for ib in range(N_IB)]
        # the 64s denominator column (the Pool engine owns SBUF memsets)
        for ch in range(N_CH):
            nc.gpsimd.memset(wT_ch[ch][:, :, C:C + 1], WS)

        with tc.tile_pool(name="P1", bufs=2, space="PSUM") as P1, \
             tc.tile_pool(name="hps", bufs=4, space="PSUM") as hps, \
             tc.tile_pool(name="ppool", bufs=4) as ppool, \
             tc.tile_pool(name="opool", bufs=6) as opool, \
             tc.tile_pool(name="rpool", bufs=8) as rpool:

            hh_ps_of = {}

            def emit_qconv(ib):
                ps = P1.tile([128, 2, 512], F32, tag="w", name=f"psq{ib}")
                for mt in range(N_KT):
                    nc.tensor.matmul(
                        ps[:, mt, :],
                        lhsT=qwT[:, :, mt * 128:(mt + 1) * 128],
                        rhs=x8_ch[ib][:, :, :],
                        start=True, stop=True, perf_mode=DR,
                    )
                for mt in range(N_KT):
                    nc.vector.tensor_scalar_add(
                        q_ch[ib][:, mt, :], ps[:, mt, :], qb[:, mt:mt + 1])

            def emit_mm1_exp(ib, pair):
                """Scores + exp for one key-tile PAIR of query block ib."""
                ch, pp = divmod(pair, 2)
                att = P1.tile([128, 2, 512], F32, tag="w",
                              name=f"att{ib}_{pair}")
                for h in range(2):
                    jj = pp * 2 + h
                    nc.tensor.matmul(
                        att[:, h, :],
                        lhsT=k_ch[ch][:, :, jj * 128:(jj + 1) * 128],
                        rhs=q_ch[ib][:, :, :],
                        start=True, stop=True, perf_mode=DR,
                    )
                pt = ppool.tile([128, 2, 512], FP8, tag="pT",
                                name=f"pt{ib}_{pair}")
                nc.scalar.activation(out=pt[:, :, :], in_=att[:, :, :],
                                     func=AF.Exp, scale=ESCALE)
                return pt

            def emit_mm2(ib, pair, pt):
                ch, pp = divmod(pair, 2)
                for isl in range(4):
                    nc.tensor.matmul(
                        hh_ps_of[ib][isl],
                        lhsT=pt[:, :, isl * 128:(isl + 1) * 128],
                        rhs=wT_ch[ch][:, pp * 2:pp * 2 + 2, :],
                        start=(pair == 0),
                        stop=(pair == N_PAIR - 1),
                        perf_mode=DR,
                    )

            def emit_tail(ib):
                """Normalize + residual + store, i-major, for block ib."""
                for isl in range(4):
                    rc = rpool.tile([128, 1], F32, tag="rc", name=f"rc{ib}_{isl}")
                    nc.vector.reciprocal(rc, hh_ps_of[ib][isl][:, C:C + 1])
                    ob = opool.tile([128, C], F32, tag="ob",
                                    name=f"ob{ib}_{isl}")
                    nc.vector.scalar_tensor_tensor(
                        ob, in0=hh_ps_of[ib][isl][:, 0:C], scalar=rc,
                        in1=xr_ib[ib][:, isl, :],
                        op0=mybir.AluOpType.mult, op1=mybir.AluOpType.add,
                    )
                    r0 = ib * 512 + isl * 128
                    nc.sync.dma_start(out=out_d[r0:r0 + 128, :], in_=ob)

            # ---- conv phase, interleaved with query block 0's attention ----
            # software-pipelined: mm1+exp of pair N+1 issues on the PE before
            # mm2 of pair N, so the PE computes scores while the ACT
            # exponentiates and mm2 starts the moment its p-tile is ready.
            hh_ps_of[0] = [hps.tile([128, CV], F32, tag="hh",
                                    name=f"hh_ps0_{isl}") for isl in range(4)]
            prev = None

            def emit_pair_piped(ib, pair):
                nonlocal prev
                pt = emit_mm1_exp(ib, pair)
                if prev is not None:
                    emit_mm2(prev[0], prev[1], prev[2])
                prev = (ib, pair, pt)

            for ch in range(N_CH):
                ps = P1.tile([128, 2, 512], F32, tag="w", name=f"psk{ch}")
                for mt in range(N_KT):
                    nc.tensor.matmul(
                        ps[:, mt, :],
                        lhsT=kwT[:, :, mt * 128:(mt + 1) * 128],
                        rhs=x8_ch[ch][:, :, :],
                        start=True, stop=True, perf_mode=DR,
                    )
                nc.vector.tensor_copy(k_ch[ch][:, :, :], ps[:, :, :])
                if ch == 0:
                    emit_qconv(0)
                for jjh in range(2):
                    ps = P1.tile([128, 2, 512], F32, tag="w",
                                 name=f"psw{ch}_{jjh}")
                    for h in range(2):
                        jj = jjh * 2 + h
                        nc.tensor.matmul(
                            ps[:, h, 0:C],
                            lhsT=x8_ch[ch][:, :, jj * 128:(jj + 1) * 128],
                            rhs=wwT[:, :, :],
                            start=True, stop=True, perf_mode=DR,
                        )
                    nc.vector.tensor_copy(
                        wT_ch[ch][:, jjh * 2:jjh * 2 + 2, 0:C], ps[:, :, 0:C])
                # attention rides right behind each chunk's convs
                for pair in range(2 * ch, 2 * ch + 2):
                    emit_pair_piped(0, pair)

            # ---- remaining query blocks ----
            pending = 0
            for ib in range(1, N_IB):
                emit_qconv(ib)
                hh_ps_of[ib] = [hps.tile([128, CV], F32, tag="hh",
                                         name=f"hh_ps{ib}_{isl}")
                                for isl in range(4)]
                for pair in range(N_PAIR):
                    emit_pair_piped(ib, pair)
                    if pending is not None and pair == 1:
                        emit_tail(pending)
                        pending = None
                pending = ib
            emit_mm2(prev[0], prev[1], prev[2])
            emit_tail(pending)

    nc.compile()
    return nc


_NC_CACHE = None


def _get_nc():
    global _NC_CACHE
    if _NC_CACHE is None:
        _NC_CACHE = build_nc()
    return _NC_CACHE


def make_in_maps(x, temb, q_w, q_b, k_w, k_b, v_w, v_b, p_w, p_b, t_w, t_b):
    xf = np.asarray(x, np.float32).reshape(B, C, HW)
    temb = np.asarray(temb, np.float32)
    fp8 = ml_dtypes.float8_e4m3
    # host-side algebraic folds
    t = temb @ np.asarray(t_w, np.float32).T + np.asarray(t_b, np.float32)
    pw = np.asarray(p_w, np.float32)
    Ww = pw @ np.asarray(v_w, np.float32)
    rbias = np.asarray(p_b, np.float32) + pw @ np.asarray(v_b, np.float32)
    common = {
        "qwT": np.ascontiguousarray(np.asarray(q_w, np.float32).T * WS).astype(fp8),
        "kwT": np.ascontiguousarray(np.asarray(k_w, np.float32).T * WS).astype(fp8),
        "wwT": np.ascontiguousarray(Ww.T * WS).astype(fp8),
    }
    in_maps = []
    for core in range(N_CORES):
        b, h = divmod(core, 2)
        m = dict(common)
        # rotate so this core's query half occupies columns 0..NQ-1;
        # key order is irrelevant (softmax sums over keys).
        xr = xf[b] if h == 0 else np.concatenate(
            [xf[b][:, NQ:], xf[b][:, :NQ]], axis=1)
        # chunk-contiguous: [N_CH, C, 512] flattened, so each 128KB chunk
        # is one linear DRAM span (fast DMA descriptors)
        m["x8"] = np.ascontiguousarray(
            xr.reshape(C, N_CH, 512).transpose(1, 0, 2)).reshape(
                N_CH * C, 512).astype(fp8)
        # residual, pre-biased and transposed to the i-major store layout
        m["xr"] = np.ascontiguousarray((xr[:, :NQ] + rbias[:, None]).T)
        m["qb"] = (WS * (np.asarray(q_b, np.float32) + t[b])).reshape(C, 1)
        in_maps.append(m)
    return in_maps


def run(in_maps, trace=False):
    nc = _get_nc()
    return run_bass_kernel_spmd(nc, in_maps, core_ids=list(range(N_CORES)),
                                trace=trace)


def kernel(**inputs):
    in_maps = make_in_maps(**inputs)
    res = run(in_maps)
    out = np.empty((B, C, HW), np.float32)
    for core in range(N_CORES):
        b, h = divmod(core, 2)
        out[b, :, h * NQ:(h + 1) * NQ] = res.results[core]["out"].T
    return out.reshape(B, C, H, W)
